# revision 7
# baseline (speedup 1.0000x reference)
"""ConvLSTM (2-layer, HID=64, 64x64, T=16, B=16) Trainium2 Bass kernel.

Sharding: data-parallel over batch B=16 -> 2 per NeuronCore across 8 cores;
weights/biases replicated; the sequential T-loop runs locally per core.

v4: mixed-precision convs.
- i,f-gate convs run in fp8e4 DoubleRow mode (K=2x per matmul at the bf16
  column rate): shift-offset pairs come from explicit k-subtile slabs in
  fp8 tiles f1 ([h0;h1] at base and (0,1)) and f1b (at (0,2) and (1,2));
  cell0 reads partition range [0:64] of the same tiles. The x-term and the
  (2,2)-offset term ride existing bf16 tiles with x8192-scaled bf16 weights
  accumulating into the same PSUM group. fp8 operands are scaled (w x64,
  h x128, x-term weights x8192) and unscaled in the sigmoid via ACT scale.
- o,g-gate convs stay bf16 (g-gate precision): cell0 as 5 matmuls per
  bank-pass (x9h K=73 incl h@(2,2), 3 offset-pairs vs hh0, 1 pair vs hbB),
  cell1 as 9 (K=128 exactly packed).
- h->fp8 conversion and fp8 fanout copies run on the idle gpsimd engine
  (own DMA ring); bf16 fanout copies on the sync ring, all overlapped
  under cell1/cell0 conv matmuls.

Within a cell, all conv matmuls are emitted before any state write so
Tile's program-order dependency tracking sees the in-place h updates
correctly; single-row seam overlaps (which Tile's subtile tracker misses)
get explicit dependency edges.
"""
import numpy as np
import ml_dtypes
import concourse.tile as tile
from concourse import mybir, bacc
from concourse.bass import _add_dep_helper
from concourse.bass_utils import run_bass_kernel_spmd

F32 = mybir.dt.float32
BF16 = mybir.dt.bfloat16
FP8 = mybir.dt.float8e4
DR = mybir.MatmulPerfMode.DoubleRow
SIG = mybir.ActivationFunctionType.Sigmoid
TANH = mybir.ActivationFunctionType.Tanh
RELU = mybir.ActivationFunctionType.Relu

N_CORES = 8
B_LOC = 2
H = W = 64
HP = WP = 66
EG_ROWS = 16
CH_ROWS = 8
N_EG = H // EG_ROWS  # 4
N_MM = CH_ROWS * W  # 512
NE = 2 * N_MM  # 1024 elems per elementwise chunk

SW = 64.0    # fp8 weight scale
SH = 128.0   # fp8 h scale
SP_ = SW * SH  # product scale for the i,f PSUM group


def _build(T=16):
    nc = bacc.Bacc("TRN2", target_bir_lowering=False, debug=False, num_devices=N_CORES)

    x9_d = nc.dram_tensor("x9", [T, 9, B_LOC, H, W], BF16, kind="ExternalInput").ap()
    # bf16 og weights
    w0p_d = nc.dram_tensor("w0p", [128, 3, 128], BF16, kind="ExternalInput").ap()
    w0x_d = nc.dram_tensor("w0x", [73, 128], BF16, kind="ExternalInput").ap()
    w0bq_d = nc.dram_tensor("w0bq", [128, 128], BF16, kind="ExternalInput").ap()
    w1_d = nc.dram_tensor("w1t", [128, 9, 128], BF16, kind="ExternalInput").ap()
    # bf16 if weights (x + (2,2) terms, x8192-scaled)
    w0ix_d = nc.dram_tensor("w0ix", [73, 128], BF16, kind="ExternalInput").ap()
    w1i22_d = nc.dram_tensor("w1i22", [128, 128], BF16, kind="ExternalInput").ap()
    # fp8 if weights (x64-scaled)
    w0fp_d = nc.dram_tensor("w0fp", [64, 3, 2, 128], FP8, kind="ExternalInput").ap()
    w0fq_d = nc.dram_tensor("w0fq", [64, 2, 128], FP8, kind="ExternalInput").ap()
    w1fp_d = nc.dram_tensor("w1fp", [128, 3, 2, 128], FP8, kind="ExternalInput").ap()
    w1fq_d = nc.dram_tensor("w1fq", [128, 2, 128], FP8, kind="ExternalInput").ap()
    b_d = nc.dram_tensor("bt", [128, 4], F32, kind="ExternalInput").ap()
    wh_d = nc.dram_tensor("wht", [128, 1], BF16, kind="ExternalInput").ap()
    bh_d = nc.dram_tensor("bht", [1, 1], F32, kind="ExternalInput").ap()
    y_d = nc.dram_tensor("y", [B_LOC, H * W], F32, kind="ExternalOutput").ap()

    with tile.TileContext(nc) as tc:
        with tc.tile_pool(name="state", bufs=1) as state, \
                tc.tile_pool(name="work", bufs=2) as work, \
                tc.tile_pool(name="psp", bufs=2, space="PSUM") as psp:
            # bf16 og-conv state
            hh0 = state.tile([128, B_LOC, HP, WP], BF16)   # [h0 ; h0@(0,1)]
            hbB = state.tile([128, B_LOC, HP, WP], BF16)   # [h0@(0,2) ; h0@(1,2)]
            x9h = state.tile([73, B_LOC, H, W], BF16)      # x im2col + h0@(2,2)
            inp1 = state.tile([128, B_LOC, HP, WP], BF16)  # [h0 ; h1] base
            # fp8 if-conv state, k-subtile slabs: slab0 base, slab1 shifted
            f1 = state.tile([128, B_LOC, 2, HP, WP], FP8)   # [h0;h1] @base, @(0,1)
            f1b = state.tile([128, B_LOC, 2, HP, WP], FP8)  # [h0;h1] @(0,2), @(1,2)
            # c state (f32) at partitions 64-127
            cg0 = state.tile([128, B_LOC, H * W], F32)
            cg1 = state.tile([128, B_LOC, H * W], F32)
            w0p = state.tile([128, 3, 128], BF16)
            w0x = state.tile([73, 128], BF16)
            w0bq = state.tile([128, 128], BF16)
            w1t = state.tile([128, 9, 128], BF16)
            w0ix = state.tile([73, 128], BF16)
            w1i22 = state.tile([128, 128], BF16)
            w0fp = state.tile([64, 3, 2, 128], FP8)
            w0fq = state.tile([64, 2, 128], FP8)
            w1fp = state.tile([128, 3, 2, 128], FP8)
            w1fq = state.tile([128, 2, 128], FP8)
            b_sb = state.tile([128, 4], F32)
            whT = state.tile([128, 1], BF16)
            bh_sb = state.tile([1, 1], F32)

            for dst, src in ((w0p, w0p_d), (w0x, w0x_d), (w0bq, w0bq_d),
                             (w1t, w1_d), (w0ix, w0ix_d), (w1i22, w1i22_d),
                             (w0fp, w0fp_d), (w0fq, w0fq_d), (w1fp, w1fp_d),
                             (w1fq, w1fq_d), (b_sb, b_d), (whT, wh_d),
                             (bh_sb, bh_d)):
                nc.sync.dma_start(out=dst, in_=src)

            nc.vector.memset(hh0.bitcast(mybir.dt.uint16), 0)
            nc.vector.memset(hbB.bitcast(mybir.dt.uint16), 0)
            nc.vector.memset(x9h.bitcast(mybir.dt.uint16), 0)
            nc.vector.memset(inp1.bitcast(mybir.dt.uint16), 0)
            nc.vector.memset(f1.bitcast(mybir.dt.uint8), 0)
            nc.vector.memset(f1b.bitcast(mybir.dt.uint8), 0)
            nc.vector.memset(cg0[64:128], 0.0)
            nc.vector.memset(cg1[64:128], 0.0)

            nc.sync.dma_start(out=x9h[0:9], in_=x9_d[0])

            h0w = {}
            h1w = {}
            mm_info = {}

            def conv0(t):
                psums = {}
                for b in range(B_LOC):
                    for eg in range(N_EG):
                        p_if = psp.tile([128, 2, N_MM], F32, tag="pif", name=f"pif_{t}_0_{b}_{eg}")
                        p_og = psp.tile([128, 2, N_MM], F32, tag="pog", name=f"pog_{t}_0_{b}_{eg}")
                        psums[(b, eg)] = (p_if, p_og)
                        for half in range(2):
                            r0 = eg * EG_ROWS + half * CH_ROWS
                            # i,f group: 4 fp8-DR + 1 bf16 (x + h@(2,2))
                            for dy in range(3):
                                nc.tensor.matmul(
                                    p_if[:, half], lhsT=w0fp[:, dy],
                                    rhs=f1[0:64, b, 0:2, r0 + dy:r0 + dy + CH_ROWS, 0:W],
                                    start=(dy == 0), stop=False, perf_mode=DR)
                            nc.tensor.matmul(
                                p_if[:, half], lhsT=w0fq,
                                rhs=f1b[0:64, b, 0:2, r0:r0 + CH_ROWS, 0:W],
                                start=False, stop=False, perf_mode=DR)
                            nc.tensor.matmul(
                                p_if[:, half], lhsT=w0ix,
                                rhs=x9h[0:73, b, r0:r0 + CH_ROWS, 0:W],
                                start=False, stop=True)
                            # o,g group: 5 bf16
                            nc.tensor.matmul(
                                p_og[:, half], lhsT=w0x,
                                rhs=x9h[0:73, b, r0:r0 + CH_ROWS, 0:W],
                                start=True, stop=False)
                            for dy in range(3):
                                nc.tensor.matmul(
                                    p_og[:, half], lhsT=w0p[:, dy],
                                    rhs=hh0[0:128, b, r0 + dy:r0 + dy + CH_ROWS, 0:W],
                                    start=False, stop=False)
                            nc.tensor.matmul(
                                p_og[:, half], lhsT=w0bq,
                                rhs=hbB[0:128, b, r0:r0 + CH_ROWS, 0:W],
                                start=False, stop=True)
                return psums

            def conv1(t):
                psums = {}
                for b in range(B_LOC):
                    for eg in range(N_EG):
                        info = mm_info.setdefault((t, b, eg), [])
                        p_if = psp.tile([128, 2, N_MM], F32, tag="pif", name=f"pif_{t}_1_{b}_{eg}")
                        p_og = psp.tile([128, 2, N_MM], F32, tag="pog", name=f"pog_{t}_1_{b}_{eg}")
                        psums[(b, eg)] = (p_if, p_og)
                        for half in range(2):
                            r0 = eg * EG_ROWS + half * CH_ROWS
                            # i,f group: 4 fp8-DR + 1 bf16 (2,2)
                            for dy in range(3):
                                nc.tensor.matmul(
                                    p_if[:, half], lhsT=w1fp[:, dy],
                                    rhs=f1[0:128, b, 0:2, r0 + dy:r0 + dy + CH_ROWS, 0:W],
                                    start=(dy == 0), stop=False, perf_mode=DR)
                            nc.tensor.matmul(
                                p_if[:, half], lhsT=w1fq,
                                rhs=f1b[0:128, b, 0:2, r0:r0 + CH_ROWS, 0:W],
                                start=False, stop=False, perf_mode=DR)
                            m22 = nc.tensor.matmul(
                                p_if[:, half], lhsT=w1i22,
                                rhs=inp1[0:128, b, r0 + 2:r0 + 2 + CH_ROWS, 2:2 + W],
                                start=False, stop=True)
                            if half == 1 and (t, b, eg + 1) in h0w:
                                _add_dep_helper(m22.ins, h0w[(t, b, eg + 1)], reason="if22 seam RAW up")
                            # o,g group: 9 bf16
                            for off in range(9):
                                dy, dx = off // 3, off % 3
                                mo = nc.tensor.matmul(
                                    p_og[:, half], lhsT=w1t[:, off],
                                    rhs=inp1[0:128, b, r0 + dy:r0 + dy + CH_ROWS, dx:dx + W],
                                    start=(off == 0), stop=(off == 8))
                                if off == 8:
                                    info.append(mo.ins)
                                if half == 0 and dy == 0 and (t, b, eg - 1) in h0w:
                                    _add_dep_helper(mo.ins, h0w[(t, b, eg - 1)], reason="h0 seam RAW dn")
                                if half == 1 and dy == 2 and (t, b, eg + 1) in h0w:
                                    _add_dep_helper(mo.ins, h0w[(t, b, eg + 1)], reason="h0 seam RAW up")
                return psums

            def elem(cell, t, psums):
                cg = cg0 if cell == 0 else cg1
                bcol = 2 * cell
                for b in range(B_LOC):
                    for eg in range(N_EG):
                        p_if, p_og = psums[(b, eg)]
                        pif_f = p_if.rearrange("p a b -> p (a b)")
                        pog_f = p_og.rearrange("p a b -> p (a b)")
                        cols = slice(eg * EG_ROWS * W, (eg + 1) * EG_ROWS * W)
                        cseg = cg[64:128, b, cols]
                        if_h = work.tile([128, NE], F32, tag="ifh", name=f"ifh_{t}_{cell}_{b}_{eg}")
                        # P: [g_h | t5 ; m1], Q: [o_h ; m2]
                        P = work.tile([128, NE], F32, tag="P", name=f"P_{t}_{cell}_{b}_{eg}")
                        Q = work.tile([128, NE], F32, tag="Q", name=f"Q_{t}_{cell}_{b}_{eg}")
                        nc.scalar.activation(out=if_h, in_=pif_f, func=SIG,
                                             bias=b_sb[:, bcol:bcol + 1], scale=1.0 / SP_)
                        nc.scalar.activation(out=P[0:64], in_=pog_f[64:128], func=TANH,
                                             bias=b_sb[64:128, bcol + 1:bcol + 2])
                        nc.scalar.activation(out=Q[0:64], in_=pog_f[0:64], func=SIG,
                                             bias=b_sb[0:64, bcol + 1:bcol + 2])
                        nc.vector.tensor_mul(P[64:128], if_h[64:128], cseg)
                        nc.vector.tensor_mul(Q[64:128], if_h[0:64], P[0:64])
                        nc.vector.tensor_add(cseg, P[64:128], Q[64:128])
                        nc.scalar.activation(out=P[0:64], in_=cseg, func=TANH)
                        rows = slice(1 + eg * EG_ROWS, 1 + (eg + 1) * EG_ROWS)
                        if cell == 0:
                            hdst = inp1[0:64, b, rows, 1:1 + W]
                        else:
                            hdst = inp1[64:128, b, rows, 1:1 + W]
                        hw = nc.vector.tensor_mul(hdst, Q[0:64], P[0:64])
                        (h0w if cell == 0 else h1w)[(t, b, eg)] = hw.ins
                        if cell == 1:
                            for dg in (-1, 1):
                                if (t, b, eg + dg) in mm_info:
                                    for mm in mm_info[(t, b, eg + dg)]:
                                        _add_dep_helper(hw.ins, mm, reason="h1 seam WAR")
                    lo, hi = (0, 64) if cell == 0 else (64, 128)
                    # fp8 convert (x128) + fp8 slab fanout on the gpsimd ring
                    nc.gpsimd.tensor_scalar_mul(
                        f1[lo:hi, b, 0, 1:1 + H, 1:1 + W],
                        inp1[lo:hi, b, 1:1 + H, 1:1 + W], SH)
                    nc.gpsimd.dma_start(out=f1[lo:hi, b, 1, 1:1 + H, 0:W],
                                        in_=f1[lo:hi, b, 0, 1:1 + H, 1:1 + W])
                    nc.gpsimd.dma_start(out=f1b[lo:hi, b, 0, 1:1 + H, 0:W - 1],
                                        in_=f1[lo:hi, b, 0, 1:1 + H, 2:1 + W])
                    nc.gpsimd.dma_start(out=f1b[lo:hi, b, 1, 0:H, 0:W - 1],
                                        in_=f1[lo:hi, b, 0, 1:1 + H, 2:1 + W])
                    if cell == 0:
                        # bf16 fanout for the o,g convs (sync ring)
                        nc.sync.dma_start(out=hh0[0:64, b, 1:1 + H, 1:1 + W],
                                          in_=inp1[0:64, b, 1:1 + H, 1:1 + W])
                        nc.sync.dma_start(out=hh0[64:128, b, 1:1 + H, 0:W],
                                          in_=inp1[0:64, b, 1:1 + H, 1:1 + W])
                        nc.sync.dma_start(out=x9h[9:73, b, 0:H - 1, 0:W - 1],
                                          in_=inp1[0:64, b, 2:1 + H, 2:1 + W])
                        nc.sync.dma_start(out=hbB[0:64, b, 1:1 + H, 0:W - 1],
                                          in_=inp1[0:64, b, 1:1 + H, 2:1 + W])
                        nc.sync.dma_start(out=hbB[64:128, b, 0:H, 0:W - 1],
                                          in_=inp1[0:64, b, 1:1 + H, 2:1 + W])

            for t in range(T):
                psums0 = conv0(t)
                if t + 1 < T:
                    nc.sync.dma_start(out=x9h[0:9], in_=x9_d[t + 1])
                elem(0, t, psums0)
                psums1 = conv1(t)
                elem(1, t, psums1)

            for b in range(B_LOC):
                for ch in range(H // CH_ROWS):
                    p_h = psp.tile([1, N_MM], F32, tag="pif", name=f"ph_{b}_{ch}")
                    rhs = inp1[64:128, b, 1 + ch * CH_ROWS:1 + (ch + 1) * CH_ROWS, 1:1 + W]
                    mh = nc.tensor.matmul(p_h, lhsT=whT[64:128], rhs=rhs, start=True, stop=True)
                    if (T - 1, b, ch // 2) in h1w:
                        _add_dep_helper(mh.ins, h1w[(T - 1, b, ch // 2)], reason="head RAW")
                    h_out = work.tile([1, N_MM], F32, tag="ho", name=f"ho_{b}_{ch}")
                    nc.scalar.activation(out=h_out, in_=p_h, func=RELU, bias=bh_sb[0:1, 0:1])
                    nc.sync.dma_start(out=y_d[b:b + 1, ch * N_MM:(ch + 1) * N_MM], in_=h_out)

    nc.compile()
    return nc


def _prep_inputs(x, w0, b0, w1, b1, wh, bh):
    bf16 = ml_dtypes.bfloat16
    fp8 = ml_dtypes.float8_e4m3fn
    x = np.asarray(x, np.float32)
    B, T = x.shape[0], x.shape[1]
    bl = B // N_CORES

    def q8(a):
        return np.clip(np.asarray(a, np.float32), -448, 448).astype(fp8)

    w0 = np.asarray(w0, np.float32).reshape(2, 128, 65, 3, 3)  # [g, m, k, dy, dx]
    w1 = np.asarray(w1, np.float32).reshape(2, 128, 128, 3, 3)

    # --- bf16 og weights (g=1: [o;g]) ---
    w0p = np.zeros((128, 3, 128), np.float32)
    for dy in range(3):
        w0p[0:64, dy] = w0[1, :, 1:65, dy, 0].T
        w0p[64:128, dy] = w0[1, :, 1:65, dy, 1].T
    w0bq = np.zeros((128, 128), np.float32)
    w0bq[0:64] = w0[1, :, 1:65, 0, 2].T
    w0bq[64:128] = w0[1, :, 1:65, 1, 2].T
    w0x = np.zeros((73, 128), np.float32)
    for o in range(9):
        w0x[o] = w0[1, :, 0, o // 3, o % 3]
    w0x[9:73] = w0[1, :, 1:65, 2, 2].T
    w1t = np.zeros((128, 9, 128), np.float32)
    for o in range(9):
        w1t[:, o] = w1[1, :, :, o // 3, o % 3].T

    # --- bf16 if weights (x + (2,2) terms), x SP_ scale ---
    w0ix = np.zeros((73, 128), np.float32)
    for o in range(9):
        w0ix[o] = w0[0, :, 0, o // 3, o % 3] * SP_
    w0ix[9:73] = w0[0, :, 1:65, 2, 2].T * SP_
    w1i22 = w1[0, :, :, 2, 2].T * SP_

    # --- fp8 if weights (x SW), k-subtile dim = dx (pairs) or dy (f1b) ---
    w0fp = np.zeros((64, 3, 2, 128), np.float32)
    for dy in range(3):
        for s in range(2):
            w0fp[:, dy, s] = w0[0, :, 1:65, dy, s].T * SW
    w0fq = np.zeros((64, 2, 128), np.float32)
    for s in range(2):
        w0fq[:, s] = w0[0, :, 1:65, s, 2].T * SW
    w1fp = np.zeros((128, 3, 2, 128), np.float32)
    for dy in range(3):
        for s in range(2):
            w1fp[:, dy, s] = w1[0, :, :, dy, s].T * SW
    w1fq = np.zeros((128, 2, 128), np.float32)
    for s in range(2):
        w1fq[:, s] = w1[0, :, :, s, 2].T * SW

    b0 = np.asarray(b0, np.float32)
    b1 = np.asarray(b1, np.float32)
    bt = np.stack([b0[0:128], b0[128:256], b1[0:128], b1[128:256]], axis=1).astype(np.float32)
    wht = np.zeros((128, 1), np.float32)
    wht[64:128, 0] = np.asarray(wh, np.float32).reshape(64)
    bht = np.array([[float(np.asarray(bh).reshape(-1)[0])]], np.float32)

    xp_all = np.zeros((B, T, HP, WP), np.float32)
    xp_all[:, :, 1:1 + H, 1:1 + W] = x[:, :, 0]

    common = {"w0p": w0p.astype(bf16), "w0x": w0x.astype(bf16),
              "w0bq": w0bq.astype(bf16), "w1t": w1t.astype(bf16),
              "w0ix": w0ix.astype(bf16), "w1i22": np.ascontiguousarray(w1i22).astype(bf16),
              "w0fp": q8(w0fp), "w0fq": q8(w0fq),
              "w1fp": q8(w1fp), "w1fq": q8(w1fq),
              "bt": bt, "wht": wht.astype(bf16), "bht": bht}

    in_maps = []
    for c in range(N_CORES):
        xp = xp_all[c * bl:(c + 1) * bl]  # [bl, T, 66, 66]
        x9 = np.zeros((T, 9, bl, H, W), np.float32)
        for o in range(9):
            dy, dx = o // 3, o % 3
            x9[:, o] = xp[:, :, dy:dy + H, dx:dx + W].transpose(1, 0, 2, 3)
        im = {"x9": np.ascontiguousarray(x9.astype(bf16))}
        im.update(common)
        in_maps.append(im)
    return in_maps


_NC_CACHE = {}


def kernel(x, w0, b0, w1, b1, wh, bh):
    x = np.asarray(x)
    B, T = x.shape[0], x.shape[1]
    if T not in _NC_CACHE:
        _NC_CACHE[T] = _build(T=T)
    nc = _NC_CACHE[T]
    in_maps = _prep_inputs(x, w0, b0, w1, b1, wh, bh)
    res = run_bass_kernel_spmd(nc, in_maps, core_ids=list(range(N_CORES)))
    bl = B // N_CORES
    out = np.zeros((B, 1, H, W), np.float32)
    for c, r in enumerate(res.results):
        out[c * bl:(c + 1) * bl, 0] = r["y"].reshape(bl, H, W)
    return out
